# revision 32
# baseline (speedup 1.0000x reference)
"""GumbelGraphNetworkClf fused Bass kernel, j-sharded across 8 trn2 cores. final.

Math (per batch b):
  pre[i,j,:] = x[j]@W_e1[:D] + x[i]@W_e1[D:] + b_e1   (= A[j] + C[i])
  n2e = relu(pre); e2e = relu(n2e @ W_e2 + b2)
  agg[j,:] = sum_i adj[i,j] * e2e[i,j,:]
  out = log_softmax(nodeMLP(agg, x), axis=-1)

Sharding: core c -> (b = c//2, j-half = c%2); each core owns 256 j columns
and all 512 i rows, so agg is complete locally -- no collective.

v3 over v2: msk (= adj*e2e) is quantized to fp8e4 (error averages out over
the 512-term i-sum; ~1.4e-3 end-to-end), which lets PE accumulate both
feature halves via DoubleRow ident-pair matmuls (0.5 cyc/row, both i-planes
in one instruction). Both agg halves live in PSUM (8 banks exactly). The
msk muls move to Pool (cost-model prices gpsimd at full rate), and the ho1
relu alternates ACT (even slots) / DVE (odd slots, lag 2).

Per slot (= i-pair), cost-model ns:
  PE  : 4x matmul py f16 (860) + 2x DoubleRow acc (110) = ~970  <- critical
  Pool: 2x TT msk fp8 [128,512]                         = ~854
  ACT : relu ho0 pair-batched (519) + relu ho1 even     = ~825 avg
  DVE : 4x TS n2e (508) + relu ho1 odd                  = ~837 avg
CoreSim: 261.2us total (baseline kernel: 899.2us); PE 94.9% busy.
Validated on hw via PJRT: rel err 1.43e-3.
"""

import sys

sys.path.insert(0, "/opt/trn_rl_repo")

import numpy as np

import concourse.bass as bass
from concourse import mybir
from concourse.bass_utils import run_bass_kernel_spmd

B, N, D, H = 4, 512, 4, 256
NJ = 256          # local j columns per core
NS = 256          # slots (i-pairs)
HT = 2
F16 = mybir.dt.float16
F32 = mybir.dt.float32
F8 = mybir.dt.float8e4
F32R = mybir.dt.float32r
AF = mybir.ActivationFunctionType
OP = mybir.AluOpType
PM = mybir.MatmulPerfMode

CHUNK_I = 16      # i rows per abc DMA chunk (= 8 slots)
NCHUNK = N // CHUNK_I

# f16 const pack (cols): A (2x256) | W2 (4x128) | Wn1 (4x128) | Wn2 (4x128)
# | Wo1h (4x128) | Wo1x (256, rows0-3) | xT (256, rows0-3) | Wo (2x4)
O_A = 0
O_W2 = O_A + HT * NJ
O_WN1 = O_W2 + 4 * 128
O_WN2 = O_WN1 + 4 * 128
O_WO1H = O_WN2 + 4 * 128
O_WO1X = O_WO1H + 4 * 128
O_XT = O_WO1X + H
O_WO = O_XT + NJ
CF16 = O_WO + HT * D
# f32 const pack (cols): b2 (2) | bn1 (2) | bn2 (2) | bo1 (2) | bo (1, rows0-3)
# | C interleaved (col O_C + 2i + t)
O_B2 = 0
O_BN1 = O_B2 + HT
O_BN2 = O_BN1 + HT
O_BO1 = O_BN2 + HT
O_BO = O_BO1 + HT
O_C = O_BO + 1
CF32 = O_C + HT * N
CF32A = O_C + 2 * 128        # consts + C for i < 128 (slots < 64)

_CACHE = {}


def build_program():
    nc = bass.Bass("TRN2", target_bir_lowering=False, num_devices=8)

    cf16_ext = nc.dram_tensor("cf16", [128, CF16], F16, kind="ExternalInput")
    cf32_ext = nc.dram_tensor("cf32", [128, CF32], F32, kind="ExternalInput")
    cf8_ext = nc.dram_tensor("cf8", [128, 2, 128], F8, kind="ExternalInput")
    cfr_ext = nc.dram_tensor("cfr", [4, 8], F32, kind="ExternalInput")
    adj_ext = nc.dram_tensor("adjr", [N, NJ], F16, kind="ExternalInput")
    out_ext = nc.dram_tensor("out", [D, NJ], F32, kind="ExternalOutput")

    from contextlib import ExitStack
    with ExitStack() as ctx:
        e = ctx.enter_context
        cf16 = e(nc.sbuf_tensor([128, CF16], F16))
        cf32 = e(nc.sbuf_tensor([128, CF32], F32))
        ipair = e(nc.sbuf_tensor("ipair", [128, 2, 128], F8))
        abc = [e(nc.sbuf_tensor(f"abc{k}", [128, CHUNK_I * NJ], F16)) for k in range(2)]
        n2e = [e(nc.sbuf_tensor(f"n2e{t}", [128, 4 * 512], F16)) for t in range(HT)]
        e2e0 = e(nc.sbuf_tensor("e2e0", [128, 4 * 512], F16))   # ho0, 2 pair bufs
        e2e1 = e(nc.sbuf_tensor("e2e1", [128, 4 * 512], F16))   # ho1, 4 slot bufs
        # msk dim1 = buf*4 + ho*2 + plane  (fp8, DoubleRow rhs slices)
        msk = e(nc.sbuf_tensor("msk", [128, 12, NJ], F8))
        aggb = [e(nc.sbuf_tensor(f"aggb{t}", [128, NJ], F16)) for t in range(HT)]
        out1 = [e(nc.sbuf_tensor(f"out1{k}", [128, NJ], F16)) for k in range(HT)]
        out2 = [e(nc.sbuf_tensor(f"out2{k}", [128, NJ], F16)) for k in range(HT)]
        out4 = [e(nc.sbuf_tensor(f"out4{k}", [128, NJ], F16)) for k in range(HT)]
        out5 = e(nc.sbuf_tensor([4, NJ], F32))
        ex = e(nc.sbuf_tensor([4, NJ], F32))
        ls = e(nc.sbuf_tensor([1, NJ], F32))
        res = e(nc.sbuf_tensor([4, NJ], F32))
        pbig = [e(nc.psum_tensor(f"pbig{k}", [128, 1024], F32)) for k in range(2)]
        pho1 = e(nc.psum_tensor("pho1", [128, 1536], F32))   # 3 banks, c%3
        pagg_b = e(nc.psum_tensor("pagg", [128, 2 * NJ], F32))  # 1 shared bank
        pagg = [pagg_b[:, t * NJ : (t + 1) * NJ] for t in range(HT)]
        d_sem = e(nc.semaphore("d_sem"))      # loop consts (cf16a+cf32+cf8 = 48)
        d2_sem = e(nc.semaphore("d2_sem"))    # MLP consts (cf16b = 16)
        d3_sem = e(nc.semaphore("d3_sem"))    # C upper half (cf32b = 16)
        d8_sem = e(nc.semaphore("d8_sem"))    # ipair fp8
        d4_sem = e(nc.semaphore("d4_sem"))    # W2 block (cf16 cols 512:1024)
        da_sem = [e(nc.semaphore(f"da{q}_sem")) for q in range(2)]  # abc chunks
        ts_sem = e(nc.semaphore("ts_sem"))    # DVE TS groups + DVE tail
        v1_sem = e(nc.semaphore("v1_sem"))    # DVE relu1 (odd contents)
        py_sem = e(nc.semaphore("py_sem"))    # PE py groups + PE tail
        id_sem = e(nc.semaphore("id_sem"))    # PE DoubleRow acc groups
        a0_sem = e(nc.semaphore("a0_sem"))    # ACT ho0 pair relu
        a1_sem = e(nc.semaphore("a1_sem"))    # ACT relu1 (even) + ACT tail
        pl_sem = e(nc.semaphore("pl_sem"))    # Pool msk TTs (2/slot)
        block = e(nc.Block())

        A_sb = [cf16[:, O_A + t * NJ : O_A + (t + 1) * NJ] for t in range(HT)]

        def W2_sb(kt, ho):
            o = O_W2 + (kt * 2 + ho) * 128
            return cf16[:, o : o + 128]

        def mlp_w(base, kt, ho):
            o = base + (kt * 2 + ho) * 128
            return cf16[:, o : o + 128]

        Wo1x_sb = cf16[0:D, O_WO1X : O_WO1X + H]
        xT_sb = cf16[0:D, O_XT : O_XT + NJ]
        Wo_sb = [cf16[:, O_WO + t * D : O_WO + (t + 1) * D] for t in range(HT)]
        def C_col(t, i):
            o = O_C + 2 * i + t
            return cf32[:, o : o + 1]
        b2_sb = [cf32[:, O_B2 + t : O_B2 + t + 1] for t in range(HT)]
        bn1_sb = [cf32[:, O_BN1 + t : O_BN1 + t + 1] for t in range(HT)]
        bn2_sb = [cf32[:, O_BN2 + t : O_BN2 + t + 1] for t in range(HT)]
        bo1_sb = [cf32[:, O_BO1 + t : O_BO1 + t + 1] for t in range(HT)]
        bo_sb = cf32[0:D, O_BO : O_BO + 1]
        cfr = e(nc.sbuf_tensor("cfr_sb", [4, 8], F32))
        ones4 = cfr[0:4, 0:1]
        ones14 = cfr[0:1, 4:8]

        # ---- semaphore milestones ----
        # d_sem:  48 after cf16+cf32+cf8
        # da_sem[q]: 16*(k//2+1) after abc chunk k (parity q = k%2)
        # ts_sem: c+1 after slot c TS group; 257 aggb0; 258 res
        # v1_sem: (o+1)/2 after DVE relu1 of odd content o
        # a1_sem: c/2+1 after ACT relu1 of even content c (128 in loop);
        #         tail 129..137
        # a0_sem: P+1 after ho0 pair P relu (128 total)
        # py_sem: c+1 after py group slot c; MLP 257..262
        # id_sem: ic+1 after DoubleRow acc pair of content ic
        # pl_sem: 2*ic+1 / 2*ic+2 after Pool msk ho0/ho1 of content ic

        @block.sync
        def _(sync):
            sync.dma_start(
                cf16[:, 0:512], cf16_ext[:, 0:512]).then_inc(d_sem, 16)
            sync.dma_start(
                cf16[:, 512:1024], cf16_ext[:, 512:1024]).then_inc(d4_sem, 16)
            sync.dma_start(ipair[:], cf8_ext[:, :, :]).then_inc(d8_sem, 16)
            for k in range(NCHUNK):
                if k >= 2:
                    sync.wait_ge(pl_sem, 16 * k - 16)   # abc[k%2] WAR
                sync.dma_start(
                    abc[k % 2][:],
                    adj_ext[None, CHUNK_I * k : CHUNK_I * (k + 1), :]
                    .broadcast_to([128, CHUNK_I, NJ]),
                ).then_inc(da_sem[k % 2], 16)
                if k == 0:
                    sync.dma_start(
                        cf16[:, 1024:CF16], cf16_ext[:, 1024:CF16]
                    ).then_inc(d2_sem, 16)
            sync.wait_ge(ts_sem, 259)
            sync.dma_start(out_ext[:, :], res[:]).then_inc(d_sem, 16)

        @block.vector
        def _(vector):
            for c in range(NS + 2):
                if c < NS:
                    if c == 0:
                        vector.wait_ge(d_sem, 32)
                    if c == 64:
                        vector.wait_ge(d3_sem, 32)
                    if c >= 4:
                        vector.wait_ge(py_sem, c - 3)     # n2e[c%4] WAR
                    nb = (c % 4) * 512
                    for t in range(HT):
                        for p in range(2):
                            mm = nc.vector.tensor_scalar(
                                n2e[t][:, nb + p * NJ : nb + (p + 1) * NJ],
                                A_sb[t], C_col(t, 2 * c + p),
                                0.0, op0=OP.add, op1=OP.max,
                            )
                    mm.then_inc(ts_sem, 1)
                if c >= 2 and (c - 2) % 2 == 1:       # relu1 odd content, lag 2
                    o = c - 2
                    vector.wait_ge(py_sem, o + 1)
                    if o >= 4:
                        vector.wait_ge(pl_sem, 2 * (o - 4) + 2)  # e2e1[o%4] WAR
                    nc.vector.tensor_scalar(
                        e2e1[:, (o % 4) * 512 : (o % 4 + 1) * 512],
                        pho1[:, (o % 3) * 512 : (o % 3 + 1) * 512],
                        b2_sb[1], 0.0, op0=OP.add, op1=OP.max,
                    ).then_inc(v1_sem, 1)
            # ---- tail ----
            vector.wait_ge(id_sem, NS)
            nc.vector.tensor_copy(aggb[0][:], pagg[0]).then_inc(ts_sem, 1)  # 257
            nc.vector.tensor_copy(aggb[1][:], pagg[1]).then_inc(ts_sem, 1)  # 258
            vector.wait_ge(py_sem, 257)
            nc.vector.tensor_scalar(
                out1[1][:], pho1[:, 512 : 512 + NJ], bn1_sb[1], 0.0,
                op0=OP.add, op1=OP.max).then_inc(v1_sem, 1)                 # 129
            vector.wait_ge(py_sem, 258)
            nc.vector.tensor_scalar(
                out2[1][:], pbig[0][:, 512 : 512 + NJ], bn2_sb[1], 0.0,
                op0=OP.add, op1=OP.max).then_inc(v1_sem, 1)                 # 130
            vector.wait_ge(py_sem, 259)
            nc.vector.tensor_scalar(
                out4[1][:], pbig[1][:, 512 : 512 + NJ], bo1_sb[1], None,
                op0=OP.add).then_inc(v1_sem, 1)                             # 131
            vector.wait_ge(py_sem, 262)
            nc.vector.tensor_tensor(
                res[:], out5[:], pbig[1][0:4, 512:768], op=OP.subtract
            ).then_inc(ts_sem, 1)                                              # 259

        @block.tensor
        def _(pe):
            for c in range(NS + 4):
                if c < NS:
                    if c == 0:
                        pe.wait_ge(d4_sem, 16)    # W2 loaded
                    pe.wait_ge(ts_sem, c + 1)
                    if c >= 4:
                        pe.wait_ge(a0_sem, c // 2 - 1)    # pbig[(c//2)%2] WAR
                    if c >= 3:                            # pho1 bank c%3 WAR
                        if (c - 3) % 2 == 0:
                            pe.wait_ge(a1_sem, (c - 3) // 2 + 1)
                        else:
                            pe.wait_ge(v1_sem, (c - 2) // 2)
                    nb = (c % 4) * 512
                    for kt in range(HT):
                        nc.tensor.matmul(
                            pbig[(c // 2) % 2][:, (c % 2) * 512 : (c % 2 + 1) * 512],
                            W2_sb(kt, 0), n2e[kt][:, nb : nb + 512],
                            start=(kt == 0), stop=(kt == 1),
                        )
                    for kt in range(HT):
                        mm = nc.tensor.matmul(
                            pho1[:, (c % 3) * 512 : (c % 3 + 1) * 512],
                            W2_sb(kt, 1), n2e[kt][:, nb : nb + 512],
                            start=(kt == 0), stop=(kt == 1),
                        )
                    mm.then_inc(py_sem, 1)
                if c >= 4:
                    ic = c - 4
                    if ic == 0:
                        pe.wait_ge(d8_sem, 16)
                    pe.wait_ge(pl_sem, 2 * ic + 2)
                    mb = (ic % 3) * 4
                    nc.tensor.matmul(
                        pagg[0], ipair[:, :, :], msk[:, mb : mb + 2, :],
                        start=(ic == 0), stop=(ic == NS - 1),
                        perf_mode=PM.DoubleRow, skip_group_check=True,
                    )
                    # shares the pagg bank: only the first ic==0 matmul may
                    # start=True (bank-granular pending-zero)
                    nc.tensor.matmul(
                        pagg[1], ipair[:, :, :], msk[:, mb + 2 : mb + 4, :],
                        start=False, stop=(ic == NS - 1),
                        perf_mode=PM.DoubleRow, skip_group_check=True,
                    ).then_inc(id_sem, 1)
            # ---- node MLP (f16 hidden, f32 softmax) ----
            pe.wait_ge(ts_sem, 258)       # aggb0+aggb1
            pe.wait_ge(d2_sem, 16)        # MLP consts
            pe.wait_ge(a1_sem, 128)       # pho1 free (ACT relu1 done)
            pe.wait_ge(v1_sem, 128)       # pho1 free (DVE relu1 done)
            for ho in range(HT):
                for t in range(HT):
                    mm = nc.tensor.matmul(
                        pho1[:, ho * 512 : ho * 512 + NJ],
                        mlp_w(O_WN1, t, ho), aggb[t][:],
                        start=(t == 0), stop=(t == 1),
                    )
            mm.then_inc(py_sem, 1)        # 257
            pe.wait_ge(a1_sem, 129)
            pe.wait_ge(v1_sem, 129)
            for ho in range(HT):
                for t in range(HT):
                    mm = nc.tensor.matmul(
                        pbig[0][:, ho * 512 : ho * 512 + NJ],
                        mlp_w(O_WN2, t, ho), out1[t][:],
                        start=(t == 0), stop=(t == 1),
                    )
            mm.then_inc(py_sem, 1)        # 258
            pe.wait_ge(a1_sem, 130)
            pe.wait_ge(v1_sem, 130)
            for ho in range(HT):
                nc.tensor.matmul(
                    pbig[1][:, ho * 512 : ho * 512 + NJ],
                    Wo1x_sb[:, ho * 128 : (ho + 1) * 128], xT_sb,
                    start=True, stop=False,
                )
                for t in range(HT):
                    mm = nc.tensor.matmul(
                        pbig[1][:, ho * 512 : ho * 512 + NJ],
                        mlp_w(O_WO1H, t, ho), out2[t][:],
                        start=False, stop=(t == 1),
                    )
            mm.then_inc(py_sem, 1)        # 259
            pe.wait_ge(a1_sem, 131)
            pe.wait_ge(v1_sem, 131)
            for t in range(HT):
                mm = nc.tensor.matmul(
                    pagg_b[0:4, 0:NJ], Wo_sb[t], out4[t][:],
                    start=(t == 0), stop=(t == 1),
                )
            mm.then_inc(py_sem, 1)        # 260
            pe.wait_ge(a1_sem, 132)
            pe.wait_ge(d3_sem, 32)        # cfr ones + cf32b
            nc.tensor.matmul(
                pho1[0:1, 0:NJ], ones4, ex[:], start=True, stop=True
            ).then_inc(py_sem, 1)         # 261
            pe.wait_ge(a1_sem, 134)
            nc.tensor.matmul(
                pbig[1][0:4, 512 : 512 + NJ], ones14, ls[:], start=True, stop=True
            ).then_inc(py_sem, 1)         # 262

        @block.scalar
        def _(scalar):
            scalar.dma_start(
                cf32[:, 0:CF32A], cf32_ext[:, 0:CF32A]).then_inc(d_sem, 16)
            scalar.dma_start(cfr[:], cfr_ext[:, :]).then_inc(d3_sem, 16)
            scalar.dma_start(
                cf32[:, CF32A:CF32], cf32_ext[:, CF32A:CF32]).then_inc(d3_sem, 16)
            for c in range(NS):
                if c % 2 == 0:            # relu1 even content
                    scalar.wait_ge(py_sem, c + 1)
                    if c >= 4:
                        scalar.wait_ge(pl_sem, 2 * (c - 4) + 2)  # e2e1[c%4] WAR
                    nc.scalar.activation(
                        e2e1[:, (c % 4) * 512 : (c % 4 + 1) * 512],
                        pho1[:, (c % 3) * 512 : (c % 3 + 1) * 512],
                        AF.Relu, bias=b2_sb[1],
                    ).then_inc(a1_sem, 1)
                else:                     # ho0 pair relu
                    P = c // 2
                    scalar.wait_ge(py_sem, c + 1)
                    if P >= 2:
                        scalar.wait_ge(pl_sem, 4 * P - 4)        # e2e0 pair WAR
                    nc.scalar.activation(
                        e2e0[:, (P % 2) * 1024 : (P % 2 + 1) * 1024],
                        pbig[P % 2][:], AF.Relu, bias=b2_sb[0],
                    ).then_inc(a0_sem, 1)
            # ---- tail (ho0 on ACT; ho1 relus on Pool) ----
            scalar.wait_ge(py_sem, 257)
            nc.scalar.activation(
                out1[0][:], pho1[:, 0:NJ],
                AF.Relu, bias=bn1_sb[0]).then_inc(a1_sem, 1)        # 129
            scalar.wait_ge(py_sem, 258)
            nc.scalar.activation(
                out2[0][:], pbig[0][:, 0:NJ],
                AF.Relu, bias=bn2_sb[0]).then_inc(a1_sem, 1)        # 130
            scalar.wait_ge(py_sem, 259)
            nc.scalar.activation(
                out4[0][:], pbig[1][:, 0:NJ],
                AF.Identity, bias=bo1_sb[0]).then_inc(a1_sem, 1)    # 131
            scalar.wait_ge(py_sem, 260)
            nc.scalar.activation(ex[:], pagg_b[0:4, 0:NJ], AF.Exp,
                                 bias=bo_sb).then_inc(a1_sem, 1)    # 132
            nc.scalar.activation(out5[:], pagg_b[0:4, 0:NJ], AF.Identity,
                                 bias=bo_sb).then_inc(a1_sem, 1)    # 133
            scalar.wait_ge(py_sem, 261)
            nc.scalar.activation(ls[:], pho1[0:1, 0:NJ],
                                 AF.Ln).then_inc(a1_sem, 1)         # 134

        @block.gpsimd
        def _(gpsimd):
            for ic in range(NS):
                gpsimd.wait_ge(a0_sem, ic // 2 + 1)
                if ic % 2 == 0:
                    gpsimd.wait_ge(a1_sem, ic // 2 + 1)
                else:
                    gpsimd.wait_ge(v1_sem, (ic + 1) // 2)
                gpsimd.wait_ge(da_sem[(ic // 8) % 2], 16 * (ic // 16 + 1))
                if ic >= 3:
                    gpsimd.wait_ge(id_sem, ic - 2)       # msk[ic%3] WAR
                mb = (ic % 3) * 4
                ab = abc[(ic // 8) % 2][:, (ic % 8) * 512 : (ic % 8 + 1) * 512]
                nc.gpsimd.tensor_mul(
                    msk[:, mb : mb + 2, :],
                    e2e0[:, (ic % 4) * 512 : (ic % 4 + 1) * 512], ab,
                ).then_inc(pl_sem, 1)
                nc.gpsimd.tensor_mul(
                    msk[:, mb + 2 : mb + 4, :],
                    e2e1[:, (ic % 4) * 512 : (ic % 4 + 1) * 512], ab,
                ).then_inc(pl_sem, 1)
    return nc


def make_in_maps(x, adj, W_e1, b_e1, W_e2, b_e2, W_n1, b_n1, W_n2, b_n2,
                 W_o1, b_o1, W_o, b_o):
    in_maps = []
    for c in range(8):
        b = c // 2
        j0 = (c % 2) * NJ
        A_loc = (x[b] @ W_e1[:D])[j0 : j0 + NJ]          # [NJ, H]
        C_full = x[b] @ W_e1[D:] + b_e1                  # [N, H]

        cf16 = np.zeros((128, CF16), np.float16)
        AT = A_loc.T.astype(np.float16)                  # [H, NJ]
        for t in range(HT):
            cf16[:, O_A + t * NJ : O_A + (t + 1) * NJ] = AT[t * 128 : (t + 1) * 128]
        for base, W in ((O_W2, W_e2), (O_WN1, W_n1), (O_WN2, W_n2),
                        (O_WO1H, W_o1[D:])):
            for kt in range(HT):
                for ho in range(HT):
                    o = base + (kt * 2 + ho) * 128
                    cf16[:, o : o + 128] = W[
                        kt * 128 : (kt + 1) * 128, ho * 128 : (ho + 1) * 128
                    ].astype(np.float16)
        cf16[0:D, O_WO1X : O_WO1X + H] = W_o1[:D].astype(np.float16)
        cf16[0:D, O_XT : O_XT + NJ] = x[b].T[:, j0 : j0 + NJ].astype(np.float16)
        for t in range(HT):
            cf16[:, O_WO + t * D : O_WO + (t + 1) * D] = W_o[
                t * 128 : (t + 1) * 128].astype(np.float16)

        cf32 = np.zeros((128, CF32), np.float32)
        CT = C_full.T.astype(np.float32)                 # [H, N]
        for t in range(HT):
            r = slice(t * 128, (t + 1) * 128)
            cf32[:, O_C + t : O_C + 2 * N : 2] = CT[r]
            cf32[:, O_B2 + t] = b_e2[r]
            cf32[:, O_BN1 + t] = b_n1[r]
            cf32[:, O_BN2 + t] = b_n2[r]
            cf32[:, O_BO1 + t] = b_o1[r]
        cf32[0:D, O_BO] = b_o

        cfr = np.zeros((4, 8), np.float32)
        cfr[:, 0] = 1.0
        cfr[0, 4:8] = 1.0

        cf8 = np.zeros((128, 2, 128), mybir.dt.np(F8))
        eye = np.eye(128, dtype=mybir.dt.np(F8))
        cf8[:, 0, :] = eye
        cf8[:, 1, :] = eye

        in_maps.append({
            "cf16": cf16,
            "cf32": cf32,
            "cf8": cf8,
            "cfr": cfr,
            "adjr": adj[b, :, j0 : j0 + NJ].astype(np.float16),
        })
    return in_maps


def run(trace=False, **inputs):
    if "nc" not in _CACHE:
        _CACHE["nc"] = build_program()
    nc = _CACHE["nc"]
    in_maps = make_in_maps(**{k: np.asarray(v) for k, v in inputs.items()})
    r = run_bass_kernel_spmd(nc, in_maps, list(range(8)), trace=trace)
    out = np.stack([
        np.concatenate([r.results[2 * b]["out"].T, r.results[2 * b + 1]["out"].T])
        for b in range(B)
    ]).astype(np.float32)
    return out, r


def kernel(**inputs):
    out, _ = run(trace=False, **inputs)
    return out

